# revision 3
# baseline (speedup 1.0000x reference)
"""DistanceNetwork (retrieval kNN cosine similarity) TRN2 Bass kernel.

reference:
    input_mag = rsqrt(max(sum(input**2), eps))              # global scalar
    support_mag = rsqrt(max(sum(support**2, axis=1), eps))  # [n]
    out[n, b, 0] = dot(support[n], input[b]) * support_mag[n] * input_mag

Shapes (hardcoded): support_set [8192, 1024] f32, input_image [2048, 1024] f32,
out [8192, 2048, 1] f32.

Sharding: support rows split across 8 cores (1024 rows / core); input_image
replicated. No collectives.

v2: both operands are converted to bf16 on the host.  The fp32 PE datapath
measures ~389ns per 512-row matmul (4-byte moving data) while bf16 streams at
~213ns, and bf16 also halves the input DMA bytes, so the kernel becomes
PE-bound at the bf16 roofline (~54.6us of matmuls/core).  Numerics: bf16
quantization of the operands gives ~2.1e-3 scale-relative error on the seed-0
data, well inside the 2e-2 gate.

Schedule highlights (per core):
  - loads (sync queue): s[kt]/x[kt][bt0] interleaved so the bt0 matmul pass
    streams kt-by-kt during the load; then x bt=1..3 tile by tile.
  - PE: bt0 kt-major (8 psum banks), then bt1..3 nt-major; the tiny ssq
    partition-reduce matmuls (2x512 bf16 rows) slot in after bt1-nt0 so they
    never stall the PE on a psum bank.
  - s^2 runs on DVE during the load phase, x^2 on ACT; all psum drains on DVE.
  - bt2/3 drains fuse the combined magnitude scale; bt0/1 drain unscaled and
    are rescaled+stored in a second DVE pass once `comb` resolves (~30us).
"""

import numpy as np
import ml_dtypes

import concourse.bass as bass
import concourse.bacc as bacc
import concourse.bass_isa as bass_isa
import concourse.tile as tile
import concourse.mybir as mybir
from concourse.bass_utils import run_bass_kernel_spmd

F32 = mybir.dt.float32
BF16 = mybir.dt.bfloat16
AF = mybir.ActivationFunctionType
ALU = mybir.AluOpType

D = 1024          # feature dim (contraction)
NS = 1024         # support rows per core
B = 2048          # query batch (replicated per core)
KT = D // 128     # 8 contraction tiles
NT = NS // 128    # 8 output-partition tiles
BT = B // 512     # 4 moving-dim chunks
EPS = 1e-10
N_CORES = 8


def _newton_rsqrt(nc, pool, a_ap, seed_ap, shape, pfx, iters=2):
    """r ~= rsqrt(a) refined from seed (1/sqrt via LUT) with Newton steps.

    r <- r * (1.5 - 0.5 * a * r * r).  All tiles [P, W] f32.
    """
    r = seed_ap
    for i in range(iters):
        t = pool.tile(shape, F32, tag=f"{pfx}_t{i}", name=f"{pfx}_t{i}")
        nc.vector.tensor_mul(t[:], r, r)
        nc.vector.tensor_mul(t[:], a_ap, t[:])
        nc.vector.tensor_scalar(
            t[:], t[:], -0.5, 1.5, op0=ALU.mult, op1=ALU.add
        )
        r2 = pool.tile(shape, F32, tag=f"{pfx}_r{i}", name=f"{pfx}_r{i}")
        nc.vector.tensor_mul(r2[:], r, t[:])
        r = r2[:]
    return r


def build_nc():
    nc = bacc.Bacc(None, target_bir_lowering=False)
    s_dram = nc.declare_dram_parameter("s_t", [D, NS], BF16, isOutput=False)
    x_dram = nc.declare_dram_parameter("x_t", [D, B], BF16, isOutput=False)
    o_dram = nc.declare_dram_parameter("out", [NS, B], F32, isOutput=True)
    ssq_dram = nc.dram_tensor("ssq_bounce", [NT, 128], F32)

    with tile.TileContext(nc) as tc:
        with (
            tc.tile_pool(name="sp", bufs=KT) as sp,
            tc.tile_pool(name="xp", bufs=KT * BT) as xp,
            tc.tile_pool(name="oh", bufs=2 * NT) as oh,      # bt0/1 held
            tc.tile_pool(name="of", bufs=8) as of,           # bt2/3 recycled
            tc.tile_pool(name="s2p", bufs=2) as s2p,
            tc.tile_pool(name="small", bufs=1) as small,
            tc.tile_pool(name="psum", bufs=8, space="PSUM") as psum,
        ):
            # ---- constants ---------------------------------------------------
            ones = small.tile([128, 1], F32)
            nc.vector.memset(ones[:], 1.0)
            ones_b = small.tile([128, 1], BF16)
            nc.vector.tensor_copy(ones_b[:], ones[:])
            # pin ACT's sqrt table set before the Square stream starts, so the
            # mid-kernel Sqrt calls don't force a ~2.7us table reload
            sq_dummy = small.tile([1, 1], F32)
            nc.scalar.activation(sq_dummy[:], ones[0:1, 0:1], AF.Sqrt)

            accs = small.tile([128, KT * BT], F32)
            s_sb = [None] * KT
            x_sb = [[None] * BT for _ in range(KT)]
            o_sb = [[None] * NT for _ in range(2)]

            def load_x(kt, bt):
                t = xp.tile([128, 512], BF16, tag="x_sb", name=f"x{kt}_{bt}")
                nc.sync.dma_start(
                    out=t[:],
                    in_=x_dram[kt * 128:(kt + 1) * 128, bt * 512:(bt + 1) * 512],
                )
                x_sb[kt][bt] = t

            # ---- input DMAs: (s, x bt=0) interleaved, then x bt=1..3 --------
            for kt in range(KT):
                t = sp.tile([128, NS], BF16, tag="s_sb", name=f"s{kt}")
                nc.sync.dma_start(
                    out=t[:], in_=s_dram[kt * 128:(kt + 1) * 128, :]
                )
                s_sb[kt] = t
                load_x(kt, 0)
            for bt in range(1, BT):
                for kt in range(KT):
                    load_x(kt, bt)

            # ---- x^2 on ACT: per-partition sums into accs columns -----------
            for bt in range(BT):
                for kt in range(KT):
                    scr = s2p.tile(
                        [128, 512], F32, tag="scr", name=f"scr{kt}_{bt}",
                        bufs=2,
                    )
                    nc.scalar.activation(
                        scr[:], x_sb[kt][bt][:], AF.Square,
                        accum_out=accs[:, (bt * KT + kt):(bt * KT + kt) + 1],
                    )

            # ---- s^2 on Pool (gpsimd) during the load phase: keeps DVE free
            # for the psum drains that gate the bt0->bt1 bank handoff --------
            s2acc = small.tile([128, NS], F32)
            nc.gpsimd.tensor_mul(s2acc[:], s_sb[0][:], s_sb[0][:])
            for kt in range(1, KT):
                tmp = s2p.tile([128, NS], F32, tag="s2t", name=f"s2t{kt}",
                               bufs=2)
                nc.gpsimd.tensor_mul(tmp[:], s_sb[kt][:], s_sb[kt][:])
                nc.gpsimd.tensor_add(s2acc[:], s2acc[:], tmp[:])
            s2bf = small.tile([128, NS], BF16)
            nc.gpsimd.tensor_copy(s2bf[:], s2acc[:])

            def main_mm(ps_ap, kt, nt, bt):
                nc.tensor.matmul(
                    ps_ap,
                    s_sb[kt][:, nt * 128:(nt + 1) * 128],
                    x_sb[kt][bt][:],
                    start=(kt == 0),
                    stop=(kt == KT - 1),
                )

            # ---- bt=0: kt-major, streams with the loads; all 8 banks --------
            ps_g0 = [
                psum.tile([128, 512], F32, tag="ps", name=f"ps0_{nt}")
                for nt in range(NT)
            ]
            for kt in range(KT):
                for nt in range(NT):
                    main_mm(ps_g0[nt][:], kt, nt, 0)

            def drain(hold_bt, nt, ps_ap):
                o = oh.tile([128, 512], F32, tag="o", name=f"o{hold_bt}_{nt}")
                nc.vector.tensor_copy(o[:], ps_ap)
                o_sb[hold_bt][nt] = o

            # nt0 drains first so bt1-nt0 (same bank) can start immediately
            drain(0, 0, ps_g0[0][:])

            # ---- bt1-nt0, then the tiny ssq partition-reduce matmuls --------
            ps_b1 = [None] * NT
            ps_b1[0] = psum.tile([128, 512], F32, tag="ps", name="ps1_0")
            for kt in range(KT):
                main_mm(ps_b1[0][:], kt, 0, 1)
            ssq_ps = [
                psum.tile([1, 512], F32, tag="ps", name=f"ssq_ps{h}")
                for h in range(2)
            ]
            for h in range(2):
                nc.tensor.matmul(
                    ssq_ps[h][:], ones_b[:],
                    s2bf[:, h * 512:(h + 1) * 512],
                    start=True, stop=True,
                )
            for nt in range(1, NT):
                drain(0, nt, ps_g0[nt][:])

            ssq_sb = small.tile([1, NS], F32)
            for h in range(2):
                nc.vector.tensor_copy(
                    ssq_sb[0:1, h * 512:(h + 1) * 512], ssq_ps[h][:]
                )

            # ---- bt=1 rest: nt-major ----------------------------------------
            for nt in range(1, NT):
                ps_b1[nt] = psum.tile([128, 512], F32, tag="ps",
                                      name=f"ps1_{nt}")
                for kt in range(KT):
                    main_mm(ps_b1[nt][:], kt, nt, 1)

            # bounce-transpose ssq [1, NS] -> [128, NT] via DRAM (ACT queue,
            # after the squares in ACT program order)
            nc.scalar.dma_start(
                out=ssq_dram[:],
                in_=ssq_sb[0:1, :].rearrange("o (t p) -> o t p", p=128),
            )
            str_sb = small.tile([128, NT], F32)
            nc.scalar.dma_start(
                out=str_sb[:], in_=ssq_dram.rearrange("t p -> p t")
            )

            # ---- bt=1 drains interleaved with the magnitude chain -----------
            for nt in range(6):
                drain(1, nt, ps_b1[nt][:])
            # s chain (str_sb dep ~bounce; xsum dep ~last square)
            smax = small.tile([128, NT], F32)
            nc.vector.tensor_scalar_max(smax[:], str_sb[:], EPS)
            s_sqrt = small.tile([128, NT], F32)
            nc.scalar.activation(s_sqrt[:], smax[:], AF.Sqrt)
            xsum = small.tile([128, 1], F32)
            nc.vector.tensor_reduce(
                xsum[:], accs[:], axis=mybir.AxisListType.X, op=ALU.add
            )
            xbc = small.tile([128, 1], F32)
            nc.gpsimd.partition_all_reduce(
                xbc[:], xsum[:], channels=128,
                reduce_op=bass_isa.ReduceOp.add,
            )
            drain(1, 6, ps_b1[6][:])
            s_seed = small.tile([128, NT], F32)
            nc.vector.reciprocal(s_seed[:], s_sqrt[:])
            srs = _newton_rsqrt(nc, small, smax[:], s_seed[:], [128, NT], "srs")
            xmax = small.tile([128, 1], F32)
            nc.vector.tensor_scalar_max(xmax[:], xbc[:], EPS)
            x_sqrt = small.tile([128, 1], F32)
            nc.scalar.activation(x_sqrt[:], xmax[:], AF.Sqrt)
            drain(1, 7, ps_b1[7][:])
            x_seed = small.tile([128, 1], F32)
            nc.vector.reciprocal(x_seed[:], x_sqrt[:])
            xrs = _newton_rsqrt(nc, small, xmax[:], x_seed[:], [128, 1], "xrs")
            # combined per-(partition, nt) scale = support_mag * x_mag
            comb = small.tile([128, NT], F32)
            nc.vector.tensor_scalar(
                comb[:], srs, xrs[:, 0:1], None, op0=ALU.mult
            )

            # ---- second pass: scale + store bt0/1 (stores on ACT queue) -----
            for hold_bt in range(2):
                for nt in range(NT):
                    o = o_sb[hold_bt][nt]
                    nc.vector.tensor_scalar(
                        o[:], o[:], comb[:, nt:nt + 1], None, op0=ALU.mult
                    )
                    nc.scalar.dma_start(
                        out=o_dram[nt * 128:(nt + 1) * 128,
                                   hold_bt * 512:(hold_bt + 1) * 512],
                        in_=o[:],
                    )

            # ---- bt = 2..3: nt-major, fused scale at drain, sync stores -----
            for bt in range(2, BT):
                for nt in range(NT):
                    ps = psum.tile([128, 512], F32, tag="ps",
                                   name=f"ps{bt}_{nt}")
                    for kt in range(KT):
                        main_mm(ps[:], kt, nt, bt)
                    o = of.tile([128, 512], F32, tag="of", name=f"o{bt}_{nt}")
                    nc.vector.tensor_scalar(
                        o[:], ps[:], comb[:, nt:nt + 1], None, op0=ALU.mult
                    )
                    nc.sync.dma_start(
                        out=o_dram[nt * 128:(nt + 1) * 128,
                                   bt * 512:(bt + 1) * 512],
                        in_=o[:],
                    )
    nc.compile()
    return nc


_NC_CACHE = []


def _get_nc():
    if not _NC_CACHE:
        _NC_CACHE.append(build_nc())
    return _NC_CACHE[0]


def kernel(support_set: np.ndarray, input_image: np.ndarray) -> np.ndarray:
    support_set = np.asarray(support_set, dtype=np.float32)
    input_image = np.asarray(input_image, dtype=np.float32)
    assert support_set.shape == (N_CORES * NS, D)
    assert input_image.shape == (B, D)

    s_t = np.ascontiguousarray(support_set.T).astype(ml_dtypes.bfloat16)
    x_t = np.ascontiguousarray(input_image.T).astype(ml_dtypes.bfloat16)
    in_maps = [
        {
            "s_t": np.ascontiguousarray(s_t[:, i * NS:(i + 1) * NS]),
            "x_t": x_t,
        }
        for i in range(N_CORES)
    ]
    nc = _get_nc()
    res = run_bass_kernel_spmd(nc, in_maps, core_ids=list(range(N_CORES)))
    global LAST_RESULT
    LAST_RESULT = res
    out = np.concatenate([res.results[i]["out"] for i in range(N_CORES)], axis=0)
    return out[:, :, None]


LAST_RESULT = None


# revision 7
# speedup vs baseline: 1.2594x; 1.2594x over previous
"""DistanceNetwork (retrieval kNN cosine similarity) TRN2 Bass kernel.

reference:
    input_mag = rsqrt(max(sum(input**2), eps))              # global scalar
    support_mag = rsqrt(max(sum(support**2, axis=1), eps))  # [n]
    out[n, b, 0] = dot(support[n], input[b]) * support_mag[n] * input_mag

Shapes (hardcoded): support_set [8192, 1024] f32, input_image [2048, 1024] f32,
out [8192, 2048, 1] f32.

Sharding: support rows split across 8 cores (1024 rows / core); input_image
replicated.  No collectives.

Both operands are host-converted to bf16 (~2.1e-3 scale-relative error vs the
2e-2 gate): bf16 halves input DMA bytes and the PE streams bf16 matmuls at
~222ns issue cadence per 512-row matmul (~1 row/cycle).

Measured-trace-driven schedule (per core):
  - loads on the sync queue: s[kt]/x[kt][bt0] interleaved per kt so the bt0
    matmul pass streams kt-by-kt behind the DMA; x for bt=1..3 are ONE DMA
    each ([128, KT*512] gathered across kt) because the ~0.6us per-DMA issue
    cost on the queue engine, not bandwidth, was pacing the many-small-DMA
    version.
  - PE: bt0 kt-major across all 8 psum banks, then bt1-nt0, the two tiny ssq
    partition-reduce matmuls, bt1-nt1..7 / bt2 / bt3 nt-major.
  - s^2 on DVE (fast: ~0.2us/op; gpsimd Pool measured 10x slower), x^2 on ACT
    (wide single-instruction squares for bt1..3), drains on DVE.
  - bt2/3 drains fuse the combined magnitude scale (comb resolves ~35us,
    first fused use ~46us); bt0/1 drain unscaled (x1.0 tensor_scalar) and are
    rescaled + stored via the ACT queue in a second pass.
"""

import numpy as np
import ml_dtypes

import concourse.bass as bass
import concourse.bacc as bacc
import concourse.bass_isa as bass_isa
import concourse.tile as tile
import concourse.mybir as mybir
from concourse.bass_utils import run_bass_kernel_spmd

F32 = mybir.dt.float32
BF16 = mybir.dt.bfloat16
AF = mybir.ActivationFunctionType
ALU = mybir.AluOpType

D = 1024          # feature dim (contraction)
NS = 1024         # support rows per core
B = 2048          # query batch (replicated per core)
KT = D // 128     # 8 contraction tiles
NT = NS // 128    # 8 output-partition tiles
BT = B // 512     # 4 moving-dim chunks
EPS = 1e-10
N_CORES = 8


def _newton_rsqrt(nc, pool, a_ap, seed_ap, shape, pfx, iters=2):
    """r ~= rsqrt(a) refined from seed (1/sqrt via LUT) with Newton steps.

    r <- r * (1.5 - 0.5 * a * r * r).  All tiles [P, W] f32.
    """
    r = seed_ap
    for i in range(iters):
        t = pool.tile(shape, F32, tag=f"{pfx}_t{i}", name=f"{pfx}_t{i}")
        nc.vector.tensor_mul(t[:], r, r)
        nc.vector.tensor_mul(t[:], a_ap, t[:])
        nc.vector.tensor_scalar(
            t[:], t[:], -0.5, 1.5, op0=ALU.mult, op1=ALU.add
        )
        r2 = pool.tile(shape, F32, tag=f"{pfx}_r{i}", name=f"{pfx}_r{i}")
        nc.vector.tensor_mul(r2[:], r, t[:])
        r = r2[:]
    return r


def build_nc():
    nc = bacc.Bacc(None, target_bir_lowering=False)
    s_dram = nc.declare_dram_parameter("s_t", [D, NS], BF16, isOutput=False)
    x_dram = nc.declare_dram_parameter("x_t", [D, B], BF16, isOutput=False)
    o_dram = nc.declare_dram_parameter("out", [NS, B], F32, isOutput=True)
    ssq_dram = nc.dram_tensor("ssq_bounce", [NT, 128], F32)

    with tile.TileContext(nc) as tc:
        with (
            tc.tile_pool(name="sp", bufs=KT) as sp,
            tc.tile_pool(name="xp", bufs=KT) as xp,
            tc.tile_pool(name="oh", bufs=2 * NT) as oh,      # bt0/1 held
            tc.tile_pool(name="of", bufs=8) as of,           # bt2/3 recycled
            tc.tile_pool(name="s2p", bufs=2) as s2p,
            tc.tile_pool(name="small", bufs=1) as small,
            tc.tile_pool(name="psum", bufs=8, space="PSUM") as psum,
        ):
            # ---- constants ---------------------------------------------------
            ones = small.tile([128, 1], F32)
            nc.vector.memset(ones[:], 1.0)
            ones_b = small.tile([128, 1], BF16)
            nc.vector.tensor_copy(ones_b[:], ones[:])
            # pin ACT's sqrt table set before the Square stream starts, so the
            # mid-kernel Sqrt calls don't force a ~2.7us table reload
            sq_dummy = small.tile([1, 1], F32)
            nc.scalar.activation(sq_dummy[:], ones[0:1, 0:1], AF.Sqrt)

            accs = small.tile([128, KT + 3], F32)
            s_sb = [None] * KT
            x0_sb = [None] * KT
            xr_sb = [None] * BT   # bt=1..3: [128, KT, 512]

            # ---- input DMAs: (s, x bt=0) interleaved per kt, then one DMA
            # per remaining bt (issue cost, not bandwidth, paces the queue) --
            for kt in range(KT):
                t = sp.tile([128, NS], BF16, tag="s_sb", name=f"s{kt}")
                nc.sync.dma_start(
                    out=t[:], in_=s_dram[kt * 128:(kt + 1) * 128, :]
                )
                s_sb[kt] = t
                tx = xp.tile([128, 512], BF16, tag="x_sb", name=f"x{kt}_0")
                nc.sync.dma_start(
                    out=tx[:], in_=x_dram[kt * 128:(kt + 1) * 128, 0:512]
                )
                x0_sb[kt] = tx
            for bt in range(1, BT):
                t = xp.tile([128, KT, 512], BF16, tag="xr_sb", name=f"xr{bt}",
                            bufs=3)
                nc.sync.dma_start(
                    out=t[:],
                    in_=x_dram[:, bt * 512:(bt + 1) * 512].rearrange(
                        "(t p) c -> p t c", p=128
                    ),
                )
                xr_sb[bt] = t

            def xtile(kt, bt):
                return x0_sb[kt][:] if bt == 0 else xr_sb[bt][:, kt, :]

            # ---- x^2 on ACT: per-partition sums into accs columns -----------
            for kt in range(KT):
                scr = s2p.tile([128, 512], F32, tag="scr", name=f"scr{kt}",
                               bufs=2)
                nc.scalar.activation(
                    scr[:], x0_sb[kt][:], AF.Square,
                    accum_out=accs[:, kt:kt + 1],
                )
            for bt in range(1, BT):
                scrw = s2p.tile([128, KT * 512], F32, tag="scrw",
                                name=f"scrw{bt}", bufs=1)
                nc.scalar.activation(
                    scrw[:], xr_sb[bt][:].rearrange("p t c -> p (t c)"),
                    AF.Square,
                    accum_out=accs[:, KT + bt - 1:KT + bt],
                )

            # ---- s^2 on DVE during the load phase ---------------------------
            s2acc = small.tile([128, NS], F32)
            nc.vector.tensor_mul(s2acc[:], s_sb[0][:], s_sb[0][:])
            for kt in range(1, KT):
                tmp = s2p.tile([128, NS], F32, tag="s2t", name=f"s2t{kt}",
                               bufs=2)
                nc.vector.tensor_mul(tmp[:], s_sb[kt][:], s_sb[kt][:])
                nc.vector.tensor_add(s2acc[:], s2acc[:], tmp[:])
            s2bf = small.tile([128, NS], BF16)
            nc.vector.tensor_copy(s2bf[:], s2acc[:])

            def main_mm(ps_ap, kt, nt, bt):
                nc.tensor.matmul(
                    ps_ap,
                    s_sb[kt][:, nt * 128:(nt + 1) * 128],
                    xtile(kt, bt),
                    start=(kt == 0),
                    stop=(kt == KT - 1),
                )

            # ---- bt=0: kt-major, streams with the loads; all 8 banks --------
            ps_g0 = [
                psum.tile([128, 512], F32, tag="ps", name=f"ps0_{nt}")
                for nt in range(NT)
            ]
            for kt in range(KT):
                for nt in range(NT):
                    main_mm(ps_g0[nt][:], kt, nt, 0)

            def drain_unscaled(hold_bt, nt, ps_ap):
                o = oh.tile([128, 512], F32, tag="o", name=f"o{hold_bt}_{nt}")
                nc.vector.tensor_scalar(o[:], ps_ap, 1.0, None, op0=ALU.mult)
                o_sb[hold_bt][nt] = o

            o_sb = [[None] * NT for _ in range(2)]
            for nt in range(NT):
                drain_unscaled(0, nt, ps_g0[nt][:])

            # ---- bt1-nt0 (bank0 freed first), then the ssq matmuls ----------
            ps_b1 = [None] * NT
            ps_b1[0] = psum.tile([128, 512], F32, tag="ps", name="ps1_0")
            for kt in range(KT):
                main_mm(ps_b1[0][:], kt, 0, 1)
            ssq_ps = [
                psum.tile([1, 512], F32, tag="ps", name=f"ssq_ps{h}")
                for h in range(2)
            ]
            for h in range(2):
                nc.tensor.matmul(
                    ssq_ps[h][:], ones_b[:],
                    s2bf[:, h * 512:(h + 1) * 512],
                    start=True, stop=True,
                )
            ssq_sb = small.tile([1, NS], F32)
            for h in range(2):
                nc.vector.tensor_copy(
                    ssq_sb[0:1, h * 512:(h + 1) * 512], ssq_ps[h][:]
                )

            # bounce-transpose ssq [1, NS] -> [128, NT] via DRAM (ACT queue,
            # lands after the squares in ACT program order)
            nc.scalar.dma_start(
                out=ssq_dram[:],
                in_=ssq_sb[0:1, :].rearrange("o (t p) -> o t p", p=128),
            )
            str_sb = small.tile([128, NT], F32)
            nc.scalar.dma_start(
                out=str_sb[:], in_=ssq_dram.rearrange("t p -> p t")
            )

            # ---- bt=1 rest: nt-major, drains + magnitude chain sprinkled ----
            for nt in range(1, NT):
                ps_b1[nt] = psum.tile([128, 512], F32, tag="ps",
                                      name=f"ps1_{nt}")
                for kt in range(KT):
                    main_mm(ps_b1[nt][:], kt, nt, 1)
            drain_unscaled(1, 0, ps_b1[0][:])
            for nt in range(1, 3):
                drain_unscaled(1, nt, ps_b1[nt][:])
            # s chain
            smax = small.tile([128, NT], F32)
            nc.vector.tensor_scalar_max(smax[:], str_sb[:], EPS)
            s_sqrt = small.tile([128, NT], F32)
            nc.scalar.activation(s_sqrt[:], smax[:], AF.Sqrt)
            drain_unscaled(1, 3, ps_b1[3][:])
            s_seed = small.tile([128, NT], F32)
            nc.vector.reciprocal(s_seed[:], s_sqrt[:])
            srs = _newton_rsqrt(nc, small, smax[:], s_seed[:], [128, NT], "srs")
            # x chain
            xsum = small.tile([128, 1], F32)
            nc.vector.tensor_reduce(
                xsum[:], accs[:], axis=mybir.AxisListType.X, op=ALU.add
            )
            xbc = small.tile([128, 1], F32)
            nc.gpsimd.partition_all_reduce(
                xbc[:], xsum[:], channels=128,
                reduce_op=bass_isa.ReduceOp.add,
            )
            drain_unscaled(1, 4, ps_b1[4][:])
            xmax = small.tile([128, 1], F32)
            nc.vector.tensor_scalar_max(xmax[:], xbc[:], EPS)
            x_sqrt = small.tile([128, 1], F32)
            nc.scalar.activation(x_sqrt[:], xmax[:], AF.Sqrt)
            drain_unscaled(1, 5, ps_b1[5][:])
            x_seed = small.tile([128, 1], F32)
            nc.vector.reciprocal(x_seed[:], x_sqrt[:])
            xrs = _newton_rsqrt(nc, small, xmax[:], x_seed[:], [128, 1], "xrs")
            # combined per-(partition, nt) scale = support_mag * x_mag
            comb = small.tile([128, NT], F32)
            nc.vector.tensor_scalar(
                comb[:], srs, xrs[:, 0:1], None, op0=ALU.mult
            )
            for nt in range(6, NT):
                drain_unscaled(1, nt, ps_b1[nt][:])

            # ---- second pass: scale + store bt0/1 (stores on ACT queue) -----
            for hold_bt in range(2):
                for nt in range(NT):
                    o = o_sb[hold_bt][nt]
                    nc.vector.tensor_scalar(
                        o[:], o[:], comb[:, nt:nt + 1], None, op0=ALU.mult
                    )
                    nc.scalar.dma_start(
                        out=o_dram[nt * 128:(nt + 1) * 128,
                                   hold_bt * 512:(hold_bt + 1) * 512],
                        in_=o[:],
                    )

            # ---- bt = 2..3: nt-major, fused scale at drain, sync stores -----
            for bt in range(2, BT):
                for nt in range(NT):
                    ps = psum.tile([128, 512], F32, tag="ps",
                                   name=f"ps{bt}_{nt}")
                    for kt in range(KT):
                        main_mm(ps[:], kt, nt, bt)
                    o = of.tile([128, 512], F32, tag="of", name=f"o{bt}_{nt}")
                    nc.vector.tensor_scalar(
                        o[:], ps[:], comb[:, nt:nt + 1], None, op0=ALU.mult
                    )
                    nc.sync.dma_start(
                        out=o_dram[nt * 128:(nt + 1) * 128,
                                   bt * 512:(bt + 1) * 512],
                        in_=o[:],
                    )
    nc.compile()
    return nc


_NC_CACHE = []


def _get_nc():
    if not _NC_CACHE:
        _NC_CACHE.append(build_nc())
    return _NC_CACHE[0]


def kernel(support_set: np.ndarray, input_image: np.ndarray) -> np.ndarray:
    support_set = np.asarray(support_set, dtype=np.float32)
    input_image = np.asarray(input_image, dtype=np.float32)
    assert support_set.shape == (N_CORES * NS, D)
    assert input_image.shape == (B, D)

    s_t = np.ascontiguousarray(support_set.T).astype(ml_dtypes.bfloat16)
    x_t = np.ascontiguousarray(input_image.T).astype(ml_dtypes.bfloat16)
    in_maps = [
        {
            "s_t": np.ascontiguousarray(s_t[:, i * NS:(i + 1) * NS]),
            "x_t": x_t,
        }
        for i in range(N_CORES)
    ]
    nc = _get_nc()
    res = run_bass_kernel_spmd(nc, in_maps, core_ids=list(range(N_CORES)))
    global LAST_RESULT
    LAST_RESULT = res
    out = np.concatenate([res.results[i]["out"] for i in range(N_CORES)], axis=0)
    return out[:, :, None]


LAST_RESULT = None
